# revision 8
# baseline (speedup 1.0000x reference)
"""BitLinear (RMSNorm + int8 absmax activation quant + ternary absmean weight
quant + linear + rescale) on 8 Trainium2 NeuronCores.

Sharding: 2 row-groups x 4 col-groups. Each core gets half the rows of x and a
quarter of the weight rows (out_features), computes its [R/2, O/4] output
block; the host assembles the 8 blocks.

Matmul strategy: fp8e4 (e4m3) matmuls in DoubleRow perf mode (2x bf16
throughput, 256-deep contraction per instruction). The quantized activations
xq are integers in [-127, 127]; e4m3 represents them with <=4 units of
rounding error above magnitude 16. We run the main matmul on e4m3(xq) and an
exact residual-correction matmul (r = xq - e4m3(xq), an integer in [-4, 4],
exactly representable in e4m3) over the first CORR_TILES of the 16 k-tiles.
CORR_TILES=16 makes the result bit-exact; lower values trade a little
accuracy (rel err ~ 2.8e-2 * sqrt(1 - CORR_TILES/16)) for proportionally
less tensor-engine time. Ternary weights {-1,0,1} are exact in e4m3.

Host-side prep (data movement / layout only): reshape x, transpose each
core's weight slice to [d_in, o] so no on-device transpose of W is needed,
and compute the single global scalar mean(|W|) (the absmean weight scale)
so the weight quantization does not serialize behind a cross-device
AllReduce. All per-element math (rmsnorm, activation quant, weight
ternarization, matmul, rescale) runs on device. Output is stored fp16
(adds ~2e-4 relative error) and widened to f32 on the host.
"""

import sys

sys.path.insert(0, "/opt/trn_rl_repo")

import numpy as np

B, S, D_IN, D_OUT = 4, 2048, 2048, 8192
N_CORES = 8
N_R, N_O = 2, 4
R = B * S // N_R      # rows of x per core
O = D_OUT // N_O      # out cols per core
EPS = 1e-6
MAGIC = 12582912.0    # 1.5 * 2**23: fp32 add/sub round-to-nearest-even trick
CORR_TILES = 16       # k-tiles (of 16) getting the exact e4m3 residual fix


def build_nc(rows, d_in, o_cols, corr_tiles):
    """Build the SPMD bass program for one core."""
    import concourse.tile as tile
    from concourse import bacc, mybir

    f32 = mybir.dt.float32
    bf16 = mybir.dt.bfloat16
    fp16 = mybir.dt.float16
    f8 = mybir.dt.float8e4
    DR = mybir.MatmulPerfMode.DoubleRow
    P = 128
    n_rt = rows // P            # row tiles (32)
    n_kt = d_in // P            # contraction tiles (16)
    n_pair = n_kt // 2          # DR pairs (8)
    assert corr_tiles % 2 == 0
    n_cpair = corr_tiles // 2   # correction DR pairs
    NCH = 256                   # out free per DR matmul (moving free = 512)
    n_ch = o_cols // NCH        # chunks per row tile (8)
    n_bank = o_cols // 512      # psum banks per row tile (4)

    nc = bacc.Bacc("TRN2", target_bir_lowering=False, debug=False,
                   num_devices=N_CORES)

    x_d = nc.dram_tensor("x", [rows, d_in], f32, kind="ExternalInput").ap()
    wt_d = nc.dram_tensor("wT", [d_in, o_cols], f32, kind="ExternalInput").ap()
    g_d = nc.dram_tensor("gamma", [d_in], f32, kind="ExternalInput").ap()
    ws_d = nc.dram_tensor("ws", [1], f32, kind="ExternalInput").ap()
    o_d = nc.dram_tensor("out", [rows, o_cols], fp16, kind="ExternalOutput").ap()

    with tile.TileContext(nc) as tc:
        with (
            tc.tile_pool(name="cst", bufs=1) as cst,
            tc.tile_pool(name="wst", bufs=2) as wstp,     # w f32 staging
            tc.tile_pool(name="wq8", bufs=1) as wq8p,     # ternary w, fp8
            tc.tile_pool(name="xp", bufs=3) as xp,        # x f32 in
            tc.tile_pool(name="gp", bufs=2) as gp,        # x*gamma
            tc.tile_pool(name="gmp", bufs=2) as gmp,      # magic-rounded
            tc.tile_pool(name="dmp", bufs=2) as dmp,      # square dump
            tc.tile_pool(name="xqp", bufs=2) as xqp,      # xq bf16 natural
            tc.tile_pool(name="xtp", bufs=3) as xtp,      # xqT bf16
            tc.tile_pool(name="x8p", bufs=6) as x8p,      # xqT fp8
            tc.tile_pool(name="r8p", bufs=6) as r8p,      # residual fp8
            tc.tile_pool(name="stp", bufs=6) as stp,      # per-row stats
            tc.tile_pool(name="op", bufs=8) as op,        # out fp16 staging
            tc.tile_pool(name="psp", bufs=2, space="PSUM") as psp,
        ):
            # ---- constants ----
            gam = cst.tile([P, d_in], f32)
            nc.sync.dma_start(gam[:], g_d.unsqueeze(0).partition_broadcast(P))
            wsb = cst.tile([P, 1], f32)
            nc.gpsimd.dma_start(wsb[:], ws_d.unsqueeze(0).partition_broadcast(P))
            mg = cst.tile([P, 1], f32)
            nc.vector.memset(mg[:], MAGIC)
            rws = cst.tile([P, 1], f32)
            nc.vector.reciprocal(rws[:], wsb[:])
            wsc = cst.tile([P, 1], f32)
            nc.vector.tensor_scalar(wsc[:], wsb[:], 1.0 / 127.0, None,
                                    op0=mybir.AluOpType.mult)

            # ternary weights, transposed: wq8[d%128, d//128, o]
            wq8 = wq8p.tile([P, n_kt, o_cols], f8)

            def w_block(dt):
                wt = wstp.tile([P, o_cols], f32, tag="wt")
                nc.gpsimd.dma_start(wt[:], wt_d[dt * P:(dt + 1) * P, :])
                # round(w/ws) via magic add/sub; clip to [-1, 1]; cast fp8
                nc.scalar.activation(wt[:], wt[:],
                                     mybir.ActivationFunctionType.Identity,
                                     bias=mg[:], scale=rws[:])
                w1 = wstp.tile([P, o_cols], f32, tag="w1")
                nc.vector.tensor_scalar(w1[:], wt[:], MAGIC, 1.0,
                                        op0=mybir.AluOpType.subtract,
                                        op1=mybir.AluOpType.min)
                nc.gpsimd.tensor_scalar(wq8[:, dt, :], w1[:], -1.0, None,
                                        op0=mybir.AluOpType.max)

            quant_out = {}

            def x_quant(i):
                xt = xp.tile([P, d_in], f32)
                nc.sync.dma_start(xt[:], x_d[i * P:(i + 1) * P, :])
                # ss = sum(x^2) along the row (dump tile holds x^2)
                dm = dmp.tile([P, d_in], f32)
                ss = stp.tile([P, 1], f32, tag="ss")
                nc.scalar.activation(dm[:], xt[:],
                                     mybir.ActivationFunctionType.Square,
                                     accum_out=ss[:])
                # gt = x * gamma;  mx = max|gt|
                gt = gp.tile([P, d_in], f32)
                nc.gpsimd.tensor_tensor(out=gt[:], in0=xt[:], in1=gam[:],
                                        op=mybir.AluOpType.mult)
                mx = stp.tile([P, 1], f32, tag="mx")
                nc.vector.tensor_reduce(mx[:], gt[:], axis=mybir.AxisListType.X,
                                        op=mybir.AluOpType.max,
                                        apply_absolute_value=True)
                # x_scale = max(mx/rms, 1e-5); sq = 127/(rms*x_scale)
                t1 = stp.tile([P, 1], f32, tag="t1")
                nc.vector.tensor_scalar(t1[:], ss[:], 1.0 / d_in, EPS,
                                        op0=mybir.AluOpType.mult,
                                        op1=mybir.AluOpType.add)
                rms = stp.tile([P, 1], f32, tag="rms")
                nc.scalar.activation(rms[:], t1[:],
                                     mybir.ActivationFunctionType.Sqrt)
                r1 = stp.tile([P, 1], f32, tag="r1")
                nc.vector.reciprocal(r1[:], rms[:])
                xsc = stp.tile([P, 1], f32, tag="xsc")
                nc.vector.tensor_scalar(xsc[:], mx[:], r1[:], 1e-5,
                                        op0=mybir.AluOpType.mult,
                                        op1=mybir.AluOpType.max)
                d0 = stp.tile([P, 1], f32, tag="d0")
                nc.vector.tensor_tensor(out=d0[:], in0=rms[:], in1=xsc[:],
                                        op=mybir.AluOpType.mult)
                d1 = stp.tile([P, 1], f32, tag="d1")
                nc.vector.tensor_scalar(d1[:], d0[:], 1.0 / 127.0, None,
                                        op0=mybir.AluOpType.mult)
                sq = stp.tile([P, 1], f32, tag="sq")
                nc.vector.reciprocal(sq[:], d1[:])
                osc = stp.tile([P, 1], f32, tag="osc")
                nc.vector.tensor_tensor(out=osc[:], in0=xsc[:], in1=wsc[:],
                                        op=mybir.AluOpType.mult)
                # xq = round(gt * sq) via magic add/sub -> bf16
                gm = gmp.tile([P, d_in], f32)
                nc.vector.tensor_scalar(gm[:], gt[:], sq[:], MAGIC,
                                        op0=mybir.AluOpType.mult,
                                        op1=mybir.AluOpType.add)
                xq = xqp.tile([P, d_in], bf16)
                nc.vector.tensor_scalar(xq[:], gm[:], MAGIC, None,
                                        op0=mybir.AluOpType.subtract)
                xqT = xtp.tile([P, n_kt, P], bf16)
                nc.sync.dma_start_transpose(xqT[:], xq[:])
                # e4m3 main operand + exact residual for corrected k-tiles
                x8 = x8p.tile([P, n_kt, P], f8)
                nc.vector.tensor_scalar(x8[:], xqT[:], 1.0, None,
                                        op0=mybir.AluOpType.mult)
                r8 = None
                if n_cpair:
                    r8 = r8p.tile([P, 2 * n_cpair, P], f8)
                    nc.vector.tensor_tensor(
                        out=r8[:], in0=xqT[:, :2 * n_cpair, :],
                        in1=x8[:, :2 * n_cpair, :],
                        op=mybir.AluOpType.subtract)
                quant_out[i] = (x8, r8, osc)

            def x_matmul(i):
                x8, r8, osc = quant_out.pop(i)
                # matmuls: pairs outer, chunks inner; one start per bank
                pss = [psp.tile([P, 512], f32, tag=f"ps{b}", name=f"ps{b}_{i}")
                       for b in range(n_bank)]
                for t in range(n_pair):
                    for c in range(n_ch):
                        b, h = divmod(c, 2)
                        last = (t == n_pair - 1) and n_cpair == 0
                        nc.tensor.matmul(
                            pss[b][:, h * NCH:(h + 1) * NCH],
                            x8[:, 2 * t:2 * t + 2, :],
                            wq8[:, 2 * t:2 * t + 2, c * NCH:(c + 1) * NCH],
                            start=(t == 0 and h == 0), stop=last,
                            perf_mode=DR)
                for t in range(n_cpair):
                    for c in range(n_ch):
                        b, h = divmod(c, 2)
                        last = (t == n_cpair - 1)
                        nc.tensor.matmul(
                            pss[b][:, h * NCH:(h + 1) * NCH],
                            r8[:, 2 * t:2 * t + 2, :],
                            wq8[:, 2 * t:2 * t + 2, c * NCH:(c + 1) * NCH],
                            start=False, stop=last,
                            perf_mode=DR)
                # evict: scale by osc, store fp16
                for b in range(n_bank):
                    ot = op.tile([P, 512], fp16, tag="ot", name=f"ot_{i}_{b}")
                    nc.scalar.activation(ot[:], pss[b][:],
                                         mybir.ActivationFunctionType.Copy,
                                         scale=osc[:])
                    nc.gpsimd.dma_start(
                        o_d[i * P:(i + 1) * P, b * 512:(b + 1) * 512], ot[:])

            # schedule: quantize the first few x tiles interleaved with the
            # weight blocks (no matmuls yet — a matmul may only be emitted
            # after every wq8 slice it reads has been emitted), then emit
            # the deferred matmuls and stream the remaining tiles.
            LEAD = 4
            x_quant(0)
            emitted_w = 0
            for i in range(1, LEAD):
                x_quant(i)
                while emitted_w < min(n_kt, (i * n_kt) // (LEAD - 1)):
                    w_block(emitted_w)
                    emitted_w += 1
            while emitted_w < n_kt:
                w_block(emitted_w)
                emitted_w += 1
            for i in range(n_rt):
                if i + LEAD < n_rt:
                    x_quant(i + LEAD)
                x_matmul(i)

    nc.compile()
    return nc


_cache = {}


def _get_nc():
    if "nc" not in _cache:
        _cache["nc"] = build_nc(R, D_IN, O, CORR_TILES)
    return _cache["nc"]


def _in_maps(x, weight, gamma):
    X = np.ascontiguousarray(np.asarray(x, np.float32).reshape(B * S, D_IN))
    W = np.asarray(weight, np.float32)
    G = np.ascontiguousarray(np.asarray(gamma, np.float32))
    ws = np.float32(max(np.abs(W).mean(dtype=np.float64), 1e-5))
    wst = np.array([ws], np.float32)
    maps = []
    for c in range(N_CORES):
        ri, oj = divmod(c, N_O)
        maps.append({
            "x": X[ri * R:(ri + 1) * R],
            "wT": np.ascontiguousarray(W[oj * O:(oj + 1) * O, :].T),
            "gamma": G,
            "ws": wst,
        })
    return maps


def _assemble(results):
    out = np.empty((B * S, D_OUT), np.float32)
    for c in range(N_CORES):
        ri, oj = divmod(c, N_O)
        out[ri * R:(ri + 1) * R, oj * O:(oj + 1) * O] = results[c]["out"]
    return out.reshape(B, S, D_OUT)


def run(x, weight, gamma, trace=False):
    from concourse.bass_utils import run_bass_kernel_spmd

    nc = _get_nc()
    res = run_bass_kernel_spmd(nc, _in_maps(x, weight, gamma),
                               core_ids=list(range(N_CORES)), trace=trace)
    return _assemble(res.results), res


def kernel(x, weight, gamma):
    out, _ = run(x, weight, gamma)
    return out


# revision 9
# speedup vs baseline: 1.7929x; 1.7929x over previous
"""BitLinear (RMSNorm + int8 absmax activation quant + ternary absmean weight
quant + linear + rescale) on 8 Trainium2 NeuronCores.

Sharding: 2 row-groups x 4 col-groups. Each core gets half the rows of x and a
quarter of the weight rows (out_features), computes its [R/2, O/4] output
block; the host assembles the 8 blocks.

The matmul runs in bf16, which is exact here: quantized activations are
integers in [-127, 127], quantized weights are in {-1, 0, 1}, both exactly
representable in bf16, and fp32 PSUM accumulation of integer products of this
magnitude is exact. Output is stored fp16 (adds ~2e-4 relative error, well
inside the 2e-2 gate) and widened to f32 on the host.

Host-side prep (data movement / layout only): reshape x, pre-transpose each
core's weight slice to [d_in, o] so the device needs no weight transpose, and
compute the single global scalar mean(|W|) (the absmean weight scale) so
weight quantization does not serialize behind a cross-device AllReduce.
All per-element math (rmsnorm, activation quant, weight ternarization,
matmul, rescale) runs on device.

Schedule: activation quantization runs LEAD row tiles ahead of the matmul
stream so the PE never waits on the scalar/vector engines; weight
quantization is interleaved with the first x tiles at the start.
"""

import sys

sys.path.insert(0, "/opt/trn_rl_repo")

import numpy as np

B, S, D_IN, D_OUT = 4, 2048, 2048, 8192
N_CORES = 8
N_R, N_O = 2, 4
R = B * S // N_R      # rows of x per core
O = D_OUT // N_O      # out cols per core
EPS = 1e-6
MAGIC = 12582912.0    # 1.5 * 2**23: fp32 add/sub round-to-nearest-even trick


def build_nc(rows, d_in, o_cols):
    """Build the SPMD bass program for one core."""
    import concourse.tile as tile
    from concourse import bacc, mybir

    f32 = mybir.dt.float32
    bf16 = mybir.dt.bfloat16
    fp16 = mybir.dt.float16
    P = 128
    n_rt = rows // P            # row tiles (32)
    n_kt = d_in // P            # contraction tiles (16)
    NCH = 512                   # psum chunk (free dim per matmul)
    n_ch = o_cols // NCH        # chunks per row tile (4)

    nc = bacc.Bacc("TRN2", target_bir_lowering=False, debug=False,
                   num_devices=N_CORES)

    x_d = nc.dram_tensor("x", [rows, d_in], f32, kind="ExternalInput").ap()
    wt_d = nc.dram_tensor("wT", [d_in, o_cols], f32, kind="ExternalInput").ap()
    g_d = nc.dram_tensor("gamma", [d_in], f32, kind="ExternalInput").ap()
    ws_d = nc.dram_tensor("ws", [1], f32, kind="ExternalInput").ap()
    o_d = nc.dram_tensor("out", [rows, o_cols], fp16, kind="ExternalOutput").ap()

    with tile.TileContext(nc) as tc:
        with (
            tc.tile_pool(name="cst", bufs=1) as cst,
            tc.tile_pool(name="wst", bufs=2) as wstp,     # w f32 staging
            tc.tile_pool(name="wqp", bufs=1) as wqp,      # ternary w, bf16
            tc.tile_pool(name="xp", bufs=3) as xp,        # x f32 in
            tc.tile_pool(name="gp", bufs=2) as gp,        # x*gamma
            tc.tile_pool(name="gmp", bufs=2) as gmp,      # magic-rounded
            tc.tile_pool(name="dmp", bufs=1) as dmp,      # square dump
            tc.tile_pool(name="xqp", bufs=2) as xqp,      # xq bf16 natural
            tc.tile_pool(name="xtp", bufs=6) as xtp,      # xqT bf16
            tc.tile_pool(name="stp", bufs=6) as stp,      # per-row stats
            tc.tile_pool(name="op", bufs=8) as op,        # out fp16 staging
            tc.tile_pool(name="psp", bufs=2, space="PSUM") as psp,
        ):
            # ---- constants ----
            gam = cst.tile([P, d_in], f32)
            nc.sync.dma_start(gam[:], g_d.unsqueeze(0).partition_broadcast(P))
            wsb = cst.tile([P, 1], f32)
            nc.gpsimd.dma_start(wsb[:], ws_d.unsqueeze(0).partition_broadcast(P))
            mg = cst.tile([P, 1], f32)
            nc.vector.memset(mg[:], MAGIC)
            rws = cst.tile([P, 1], f32)
            nc.vector.reciprocal(rws[:], wsb[:])
            wsc = cst.tile([P, 1], f32)
            nc.vector.tensor_scalar(wsc[:], wsb[:], 1.0 / 127.0, None,
                                    op0=mybir.AluOpType.mult)

            # ternary weights, transposed: wqb[d%128, d//128, o]
            wqb = wqp.tile([P, n_kt, o_cols], bf16)

            def w_block(dt):
                wt = wstp.tile([P, o_cols], f32, tag="wt")
                nc.gpsimd.dma_start(wt[:], wt_d[dt * P:(dt + 1) * P, :])
                # round(w/ws) via magic add/sub; clip to [-1, 1]; cast bf16
                nc.scalar.activation(wt[:], wt[:],
                                     mybir.ActivationFunctionType.Identity,
                                     bias=mg[:], scale=rws[:])
                nc.vector.tensor_scalar(wt[:], wt[:], MAGIC, 1.0,
                                        op0=mybir.AluOpType.subtract,
                                        op1=mybir.AluOpType.min)
                nc.vector.tensor_scalar(wqb[:, dt, :], wt[:], -1.0, None,
                                        op0=mybir.AluOpType.max)

            quant_out = {}

            def x_quant(i):
                xt = xp.tile([P, d_in], f32)
                nc.sync.dma_start(xt[:], x_d[i * P:(i + 1) * P, :])
                # ss = sum(x^2) along the row (dump tile holds x^2)
                dm = dmp.tile([P, d_in], f32)
                ss = stp.tile([P, 1], f32, tag="ss")
                nc.scalar.activation(dm[:], xt[:],
                                     mybir.ActivationFunctionType.Square,
                                     accum_out=ss[:])
                # gt = x * gamma;  mx = max|gt|
                gt = gp.tile([P, d_in], f32)
                nc.vector.tensor_tensor(out=gt[:], in0=xt[:], in1=gam[:],
                                        op=mybir.AluOpType.mult)
                mx = stp.tile([P, 1], f32, tag="mx")
                nc.vector.tensor_reduce(mx[:], gt[:], axis=mybir.AxisListType.X,
                                        op=mybir.AluOpType.max,
                                        apply_absolute_value=True)
                # x_scale = max(mx/rms, 1e-5); sq = 127/(rms*x_scale)
                t1 = stp.tile([P, 1], f32, tag="t1")
                nc.vector.tensor_scalar(t1[:], ss[:], 1.0 / d_in, EPS,
                                        op0=mybir.AluOpType.mult,
                                        op1=mybir.AluOpType.add)
                rms = stp.tile([P, 1], f32, tag="rms")
                nc.scalar.activation(rms[:], t1[:],
                                     mybir.ActivationFunctionType.Sqrt)
                r1 = stp.tile([P, 1], f32, tag="r1")
                nc.vector.reciprocal(r1[:], rms[:])
                xsc = stp.tile([P, 1], f32, tag="xsc")
                nc.vector.tensor_scalar(xsc[:], mx[:], r1[:], 1e-5,
                                        op0=mybir.AluOpType.mult,
                                        op1=mybir.AluOpType.max)
                d0 = stp.tile([P, 1], f32, tag="d0")
                nc.vector.tensor_tensor(out=d0[:], in0=rms[:], in1=xsc[:],
                                        op=mybir.AluOpType.mult)
                d1 = stp.tile([P, 1], f32, tag="d1")
                nc.vector.tensor_scalar(d1[:], d0[:], 1.0 / 127.0, None,
                                        op0=mybir.AluOpType.mult)
                sq = stp.tile([P, 1], f32, tag="sq")
                nc.vector.reciprocal(sq[:], d1[:])
                osc = stp.tile([P, 1], f32, tag="osc")
                nc.vector.tensor_tensor(out=osc[:], in0=xsc[:], in1=wsc[:],
                                        op=mybir.AluOpType.mult)
                # xq = round(gt * sq) via magic add/sub -> bf16
                gm = gmp.tile([P, d_in], f32)
                nc.vector.tensor_scalar(gm[:], gt[:], sq[:], MAGIC,
                                        op0=mybir.AluOpType.mult,
                                        op1=mybir.AluOpType.add)
                xq = xqp.tile([P, d_in], bf16)
                nc.vector.tensor_scalar(xq[:], gm[:], MAGIC, None,
                                        op0=mybir.AluOpType.subtract)
                xqT = xtp.tile([P, n_kt, P], bf16)
                nc.sync.dma_start_transpose(xqT[:], xq[:])
                quant_out[i] = (xqT, osc)

            def x_matmul(i):
                xqT, osc = quant_out.pop(i)
                pss = [psp.tile([P, NCH], f32, tag=f"ps{c}", name=f"ps{c}_{i}")
                       for c in range(n_ch)]
                for k in range(n_kt):
                    for c in range(n_ch):
                        nc.tensor.matmul(
                            pss[c][:], xqT[:, k, :],
                            wqb[:, k, c * NCH:(c + 1) * NCH],
                            start=(k == 0), stop=(k == n_kt - 1))
                for c in range(n_ch):
                    ot = op.tile([P, NCH], fp16, tag="ot", name=f"ot_{i}_{c}")
                    nc.scalar.activation(ot[:], pss[c][:],
                                         mybir.ActivationFunctionType.Copy,
                                         scale=osc[:])
                    nc.gpsimd.dma_start(
                        o_d[i * P:(i + 1) * P, c * NCH:(c + 1) * NCH], ot[:])

            # schedule: quantize the first LEAD x tiles interleaved with the
            # weight blocks (a matmul may only be emitted after every wqb
            # slice it reads has been emitted), then stream with the
            # quantizer LEAD tiles ahead of the matmuls.
            LEAD = 4
            x_quant(0)
            emitted_w = 0
            for i in range(1, LEAD):
                x_quant(i)
                while emitted_w < min(n_kt, (i * n_kt) // (LEAD - 1)):
                    w_block(emitted_w)
                    emitted_w += 1
            while emitted_w < n_kt:
                w_block(emitted_w)
                emitted_w += 1
            for i in range(n_rt):
                if i + LEAD < n_rt:
                    x_quant(i + LEAD)
                x_matmul(i)

    nc.compile()
    return nc


_cache = {}


def _get_nc():
    if "nc" not in _cache:
        _cache["nc"] = build_nc(R, D_IN, O)
    return _cache["nc"]


def _in_maps(x, weight, gamma):
    X = np.ascontiguousarray(np.asarray(x, np.float32).reshape(B * S, D_IN))
    W = np.asarray(weight, np.float32)
    G = np.ascontiguousarray(np.asarray(gamma, np.float32))
    ws = np.float32(max(np.abs(W).mean(dtype=np.float64), 1e-5))
    wst = np.array([ws], np.float32)
    maps = []
    for c in range(N_CORES):
        ri, oj = divmod(c, N_O)
        maps.append({
            "x": X[ri * R:(ri + 1) * R],
            "wT": np.ascontiguousarray(W[oj * O:(oj + 1) * O, :].T),
            "gamma": G,
            "ws": wst,
        })
    return maps


def _assemble(results):
    out = np.empty((B * S, D_OUT), np.float32)
    for c in range(N_CORES):
        ri, oj = divmod(c, N_O)
        out[ri * R:(ri + 1) * R, oj * O:(oj + 1) * O] = results[c]["out"]
    return out.reshape(B, S, D_OUT)


def run(x, weight, gamma, trace=False):
    from concourse.bass_utils import run_bass_kernel_spmd

    nc = _get_nc()
    res = run_bass_kernel_spmd(nc, _in_maps(x, weight, gamma),
                               core_ids=list(range(N_CORES)), trace=trace)
    return _assemble(res.results), res


def kernel(x, weight, gamma):
    out, _ = run(x, weight, gamma)
    return out
